# revision 39
# baseline (speedup 1.0000x reference)
"""Trainium2 kernel for nn_BranchModel_9680856285960 (moe_routing).

Math: the reference scatters per-branch sparse weights into dense
(n_br, n_out, n_in) tensors, einsums against x, then takes a context-
gated masked sum over branches followed by relu.  Because the mask-
weighted branch sum commutes with the contraction over input features,
the whole model collapses to a 3-layer dense MLP

    out = relu(relu(x @ Weff1.T) @ Weff2.T) @ W3 + b3

where  Weff_l[o, i] = sum_{r,k} masks_l[ctx, r, o] * w_l[r, o, k]
                                * [idx_l[r, o, k] == i].

The effective-weight fold (a scatter-add over 5.6M index/value pairs) is
data-dependent element-granular addressing, which Trainium2 has no fast
engine for; it is done once on the host, and the device runs the dense
pipeline.

Two exact reductions beyond the baseline:
  * Dead-unit pruning: with 80% gate sparsity, ~10.7% of hidden units
    have ALL branches masked -> their Weff row is identically zero and
    the unit contributes nothing.  Those rows/columns are dropped
    exactly (h = relu(0) = 0), shrinking both layers' weights ~19%.
  * Weights are host-packed partition-major, grouped by output-column
    chunk, so each chunk is ONE ~1-1.6MB DMA with >=4KB-per-partition
    descriptors (HBM line rate), and compute consumes chunks as they
    land instead of waiting on dozens of small semaphore-chained DMAs.

Sharding: data-parallel over batch (8 cores x 128 rows), effective
weights replicated per core, fp16 on the wire, fp32 PSUM accumulation.
No collectives.
"""

import os
import sys
import numpy as np

for _p in ("/opt/trn_rl_repo",):
    if os.path.isdir(_p) and _p not in sys.path:
        sys.path.append(_p)

from contextlib import ExitStack

from concourse import bass, mybir
import concourse.bacc as bacc
import concourse.tile as tile
from concourse.bass_utils import run_bass_kernel_spmd
from concourse.masks import make_identity

F32 = mybir.dt.float32
F16 = mybir.dt.float16

BATCH, NIN, NH, NOUT = 1024, 784, 2000, 10
NCORES = 8
BS = BATCH // NCORES            # 128 batch rows per core
P = 128
KT1 = 6                         # full 128-row L1 k-tiles (rows 0..767)
K1 = P
KTAIL = NIN - KT1 * P           # 16-row k-tail, loaded as its own tiny tile

# Exposed for the test harness: the BassKernelResults of the last run.
LAST_RESULT = None
_CACHE = {}


def _chunks(n_cols, first):
    """Output-column chunks: small first chunk (starts PE early), the
    rest 512 wide (one PSUM bank), all multiples of 128."""
    assert n_cols % 128 == 0
    out = []
    first = min(first, n_cols)
    out.append((0, first))
    off = first
    while off < n_cols:
        w = min(512, n_cols - off)
        out.append((off, w))
        off += w
    return out


def _build_weff(w, idx, mask_row, n_in):
    """Fold masks + branch sum into a dense effective weight matrix.

    Weff[o, i] = sum_{r,k} mask_row[r, o] * w[r, o, k] * [idx[r, o, k] == i]
    """
    n_br, n_out, npb = w.shape
    acc = np.zeros(n_out * n_in, np.float64)
    base = (np.arange(n_out, dtype=np.int64) * n_in)[:, None]
    for r in range(n_br):
        flat = (base + idx[r].astype(np.int64)).ravel()
        vals = (w[r].astype(np.float64) * mask_row[r].astype(np.float64)[:, None]).ravel()
        acc += np.bincount(flat, weights=vals, minlength=n_out * n_in)
    return acc.reshape(n_out, n_in).astype(np.float32)


def _pack_chunks(wt, kt, kp, chunks):
    """Pack wt (kt*kp rows, n_cols) into the on-wire layout:
    flat[p, chunk-major: (c, t, col)] = wt[t*kp+p, c0+col], so one chunk
    is per-partition contiguous (kt * w * 2 bytes)."""
    n_cols = wt.shape[1]
    total = kt * n_cols
    out = np.zeros((kp, total), np.float16)
    pos = 0
    for (c0, w) in chunks:
        blk = wt[:, c0:c0 + w].reshape(kt, kp, w)     # [t, p, col]
        out[:, pos:pos + kt * w] = blk.transpose(1, 0, 2).reshape(kp, kt * w)
        pos += kt * w
    return out


def _mlp_body(tc, n1t, n2t, xT, xTl, w1pk, w1tlk, w2pk, w3p, b3r, out):
    nc = tc.nc
    n1, n2 = n1t * P, n2t * P
    ch1 = _chunks(n1, 512)
    ch2 = _chunks(n2, 512)

    with ExitStack() as ctx:
        const = ctx.enter_context(tc.tile_pool(name="const", bufs=1))
        wp = ctx.enter_context(tc.tile_pool(name="wslab", bufs=1))
        act = ctx.enter_context(tc.tile_pool(name="act", bufs=1))
        pacc = ctx.enter_context(tc.tile_pool(name="pacc", bufs=1, space="PSUM"))
        ptr = ctx.enter_context(tc.tile_pool(name="ptr", bufs=1, space="PSUM"))

        ident = const.tile([P, P], F16, tag="ident")
        warmt = const.tile([P, P], F16, tag="warmt")
        nc.gpsimd.memset(warmt[:], 0.0)   # ready ~2.5us before the ident iota
        make_identity(nc, ident[:])

        # Full-128-partition DMAs only (partition-split halves the SDMA
        # engine set).  Items alternate between the two HWDGE rings in
        # exact consumption order, so each ring's FIFO delivers in order
        # and the rings stay byte-balanced (they round-robin at packet
        # granularity, draining at ~equal rates).
        # Single queue for the whole weight stream: one InstDMACopy
        # already fans across all 16 SDMA engines, and a single FIFO
        # delivers in exact consumption order with zero cross-queue skew.
        def ordered_dma(sbuf_ap, dram_ap):
            nc.sync.dma_start(out=sbuf_ap, in_=dram_ap)

        # x first: stationary operand of every L1 matmul.
        xbig = const.tile([K1, KT1, BS], F16, tag="xbig")
        nc.scalar.dma_start(out=xbig[:], in_=xT)
        xts = [xbig[:, t, :] for t in range(KT1)]
        xtl = const.tile([KTAIL, BS], F16, tag="xtl")
        nc.scalar.dma_start(out=xtl[:], in_=xTl)
        w1tl = const.tile([KTAIL, n1], F16, tag="w1tl")
        nc.scalar.dma_start(out=w1tl[:], in_=w1tlk)

        b3t = const.tile([NOUT, 1], F32, tag="b3")
        nc.gpsimd.dma_start(out=b3t[:], in_=b3r)
        w3t = const.tile([P, n2t, NOUT], F16, tag="w3")
        nc.gpsimd.dma_start(out=w3t[:], in_=w3p)

        # Weight chunk slabs, issued in exact consumption order.  w2
        # chunks are split into two k-halves so the PE's per-chunk wait
        # stays well under the HAM re-throttle window.
        w1s, pos = [], 0
        for i, (c0, w) in enumerate(ch1):
            slab = wp.tile([K1, KT1, w], F16, name=f"w1s{i}", tag=f"w1s{i}")
            ordered_dma(slab[:], w1pk[:, pos:pos + KT1 * w])
            w1s.append(slab)
            pos += KT1 * w
        # w2 chunks arrive as separate k-sub-TILES.  Tile tracks
        # dependencies per tile, so a consumer matmul waits for every
        # DMA that writes its tile -- sub-chunks must be distinct tiles
        # for the t-loop to chase the stream.
        w2s, pos = [], 0
        for i, (c0, w) in enumerate(ch2):
            # narrow chunks load unsplit: k-halves would drop descriptors
            # to 3584B (sub-line-rate), stretching the stream tail
            nsplit = 2 if w >= 512 else 1
            bounds = [round(s * n1t / nsplit) for s in range(nsplit + 1)]
            subs = []
            for s in range(nsplit):
                k0, k1 = bounds[s], bounds[s + 1]
                st = wp.tile([P, k1 - k0, w], F16,
                             name=f"w2s{i}_{s}", tag=f"w2s{i}_{s}")
                ordered_dma(st[:], w2pk[:, pos + k0 * w:pos + k1 * w])
                subs.append((st, k0, k1))
            w2s.append(subs)
            pos += n1t * w

        def w2rhs(i, t):
            for st, k0, k1 in w2s[i]:
                if k0 <= t < k1:
                    return st[:, t - k0, :]
            raise IndexError

        # PE warmup: a long uninterrupted burst of dummy matmuls so the
        # HAM clock gate sees a full busy window and unthrottles the PE
        # to 2.4 GHz before the first weight chunk lands.  The DMA
        # stream is the long pole, so this costs nothing end-to-end.
        pts = [ptr.tile([P, P], F16, name=f"pt{i}", tag=f"pt{i}")
               for i in range(2)]
        psw = pacc.tile([P, P], F32, tag="psw")
        def keepalive(n):
            for _ in range(n):
                nc.tensor.matmul(psw[:], lhsT=warmt[:], rhs=warmt[:],
                                 start=True, stop=True)
        keepalive(48)

        # Activations: one tile per chunk / per k-tile.  (Dependencies
        # are per-tile: a single h1Tb tile would make every L2 matmul
        # wait on ALL 14 transpose-copies.)
        h1c = [act.tile([P, w], F16, name=f"h1c{i}", tag=f"h1c{i}")
               for i, (_, w) in enumerate(ch1)]
        h1T = [act.tile([P, P], F16, name=f"h1T{t}", tag=f"h1T{t}")
               for t in range(n1t)]
        h2r = [act.tile([P, w], F16, name=f"h2r{i}", tag=f"h2r{i}")
               for i, (_, w) in enumerate(ch2)]
        h2T = [act.tile([P, P], F16, name=f"h2T{t}", tag=f"h2T{t}")
               for t in range(n2t)]
        ps3 = ptr.tile([NOUT, P], F32, tag="ps3")
        o = act.tile([NOUT, P], F32, tag="o")
        pti = 0

        # Deferred per-chunk epilogues.  Emitting chunk i's transposes
        # AFTER chunk i+1's matmuls keeps the PE's strict in-order queue
        # from head-of-line blocking on DVE/ACT relu+copy latency.
        def l1_epilogue(i):
            # relus live on ACT only; copies on DVE only -- otherwise a
            # DVE copy waiting on a PE transpose blocks the next relu,
            # which blocks the next transpose (mutual serialization).
            nc.scalar.activation(h1c[i][:], ps1[i][:],
                                 mybir.ActivationFunctionType.Relu)

        def l1_transposes(i):
            nonlocal pti
            c0, w = ch1[i]
            for j in range(w // P):
                jg = (c0 // P) + j
                pt = pts[pti % 2]; pti += 1
                nc.tensor.transpose(pt[:], h1c[i][:, j * P:(j + 1) * P], ident[:])
                nc.vector.tensor_copy(h1T[jg][:], pt[:])

        def l2_relu(i):
            nc.scalar.activation(h2r[i][:], ps2[i][:],
                                 mybir.ActivationFunctionType.Relu)

        def l2_tail(i):
            nonlocal pti
            c0, w = ch2[i]
            for j in range(w // P):
                jg = (c0 // P) + j
                pt = pts[pti % 2]; pti += 1
                nc.tensor.transpose(pt[:], h2r[i][:, j * P:(j + 1) * P], ident[:])
                nc.vector.tensor_copy(h2T[jg][:], pt[:])
                nc.tensor.matmul(ps3[:], lhsT=w3t[:, jg, :], rhs=h2T[jg][:],
                                 start=(jg == 0), stop=(jg == n2t - 1))

        # ---- Layer 1, software-pipelined by one chunk
        ps1 = [pacc.tile([P, w], F32, name=f"ps1_{i}", tag=f"ps{i % 2}")
               for i, (_, w) in enumerate(ch1)]
        for i in range(len(ch1)):
            c0, w = ch1[i]
            for t in range(KT1):
                nc.tensor.matmul(ps1[i][:], lhsT=xts[t], rhs=w1s[i][:, t, :],
                                 start=(t == 0), stop=False)
            nc.tensor.matmul(ps1[i][:], lhsT=xtl[:], rhs=w1tl[:, c0:c0 + w],
                             start=False, stop=True)
            l1_epilogue(i)
            if i >= 1:
                l1_transposes(i - 1)
        l1_transposes(len(ch1) - 1)

        # ---- Layer 2, software-pipelined by one chunk (h2 transposes +
        # per-tile L3 accumulation trail by one chunk)
        ps2 = [pacc.tile([P, w], F32, name=f"ps2_{i}", tag=f"ps{2 + i % 2}")
               for i, (_, w) in enumerate(ch2)]
        n2c = len(ch2)
        for i in range(n2c):
            for t in range(n1t):
                nc.tensor.matmul(ps2[i][:], lhsT=h1T[t][:], rhs=w2rhs(i, t),
                                 start=(t == 0), stop=(t == n1t - 1))
            l2_relu(i)
            if i >= 1:
                l2_tail(i - 1)
        l2_tail(n2c - 1)

        nc.vector.tensor_add(o[:], ps3[:], b3t[:].to_broadcast([NOUT, P]))
        nc.sync.dma_start(out=out, in_=o[:])


def _get_program(n1t, n2t):
    key = (n1t, n2t)
    if key in _CACHE:
        return _CACHE[key]
    nc = bacc.Bacc("TRN2", target_bir_lowering=False, debug=False,
                   enable_asserts=False, enable_partition_id=False,
                   num_devices=NCORES)
    n1, n2 = n1t * P, n2t * P
    xT = nc.dram_tensor("xT", [K1, KT1, BS], F16, kind="ExternalInput").ap()
    xTl = nc.dram_tensor("xTl", [KTAIL, BS], F16, kind="ExternalInput").ap()
    w1pk = nc.dram_tensor("w1pk", [K1, KT1 * n1], F16, kind="ExternalInput").ap()
    w1tlk = nc.dram_tensor("w1tlk", [KTAIL, n1], F16, kind="ExternalInput").ap()
    w2pk = nc.dram_tensor("w2pk", [P, n1t * n2], F16, kind="ExternalInput").ap()
    w3p = nc.dram_tensor("w3p", [P, n2t, NOUT], F16, kind="ExternalInput").ap()
    b3r = nc.dram_tensor("b3r", [NOUT, 1], F32, kind="ExternalInput").ap()
    out = nc.dram_tensor("out", [NOUT, BS], F32, kind="ExternalOutput").ap()
    with tile.TileContext(nc) as tc:
        _mlp_body(tc, n1t, n2t, xT, xTl, w1pk, w1tlk, w2pk, w3p, b3r, out)
    nc.compile()
    _CACHE[key] = nc
    return nc


def kernel(x, w1, idx1, w2, idx2, masks1, masks2, W3, b3, context):
    global LAST_RESULT
    x = np.ascontiguousarray(np.asarray(x, dtype=np.float32))
    ctxi = int(np.asarray(context))

    weff1 = _build_weff(np.asarray(w1), np.asarray(idx1),
                        np.asarray(masks1)[ctxi], NIN)
    weff2 = _build_weff(np.asarray(w2), np.asarray(idx2),
                        np.asarray(masks2)[ctxi], NH)

    # Exact dead-unit pruning: units whose Weff row is identically zero
    # output relu(0)=0 and contribute nothing downstream.
    a1 = np.flatnonzero(np.abs(weff1).sum(1))
    a2 = np.flatnonzero(np.abs(weff2).sum(1))
    n1t = max(1, -(-len(a1) // P))
    n2t = max(1, -(-len(a2) // P))
    n1, n2 = n1t * P, n2t * P

    w1p = np.zeros((n1, NIN), np.float32); w1p[:len(a1)] = weff1[a1]
    w2p = np.zeros((n2, n1), np.float32)
    w2p[:len(a2), :len(a1)] = weff2[np.ix_(a2, a1)]
    W3p = np.zeros((n2, NOUT), np.float32); W3p[:len(a2)] = np.asarray(W3)[a2]

    # w1 transposed to (n_in, n1); rows 0..767 stream chunked, the
    # 16-row k-tail ships as its own tiny early tensor (no zero pad)
    w1t = np.ascontiguousarray(w1p.T)                      # (784, n1)
    w2t = w2p.T                                            # (n1, n2)

    w1pk = _pack_chunks(w1t[:KT1 * K1].astype(np.float16), KT1, K1,
                        _chunks(n1, 512))
    w1tlk = np.ascontiguousarray(w1t[KT1 * K1:].astype(np.float16))
    w2pk = _pack_chunks(w2t.astype(np.float16), n1t, P, _chunks(n2, 512))

    w3p = np.zeros((P, n2t, NOUT), np.float16)
    w3p[:, :, :] = W3p.astype(np.float16).reshape(n2t, P, NOUT).transpose(1, 0, 2)
    b3r = np.ascontiguousarray(
        np.asarray(b3, dtype=np.float32).reshape(NOUT, 1))

    try:
        import antenv.axon_hooks  # noqa: F401
    except Exception:
        os.environ.setdefault("BASS_NEVER_TRACE", "1")

    nc = _get_program(n1t, n2t)
    in_maps = []
    for c in range(NCORES):
        xs = x[c * BS:(c + 1) * BS].T.astype(np.float16)   # (784, 128)
        xT = np.ascontiguousarray(
            xs[:KT1 * K1].reshape(KT1, K1, BS).transpose(1, 0, 2))
        xTl = np.ascontiguousarray(xs[KT1 * K1:])          # (16, 128)
        in_maps.append({"xT": xT, "xTl": xTl, "w1pk": w1pk, "w1tlk": w1tlk,
                        "w2pk": w2pk, "w3p": w3p, "b3r": b3r})

    LAST_RESULT = run_bass_kernel_spmd(nc, in_maps, list(range(NCORES)))
    return np.concatenate(
        [LAST_RESULT.results[c]["out"].T for c in range(NCORES)], axis=0)


# revision 40
# speedup vs baseline: 1.0654x; 1.0654x over previous
"""Trainium2 kernel for nn_BranchModel_9680856285960 (moe_routing).

Math: the reference scatters per-branch sparse weights into dense
(n_br, n_out, n_in) tensors, einsums against x, then takes a context-
gated masked sum over branches followed by relu.  Because the mask-
weighted branch sum commutes with the contraction over input features,
the whole model collapses to a 3-layer dense MLP

    out = relu(relu(x @ Weff1.T) @ Weff2.T) @ W3 + b3

where  Weff_l[o, i] = sum_{r,k} masks_l[ctx, r, o] * w_l[r, o, k]
                                * [idx_l[r, o, k] == i].

The effective-weight fold (a scatter-add over 5.6M index/value pairs) is
data-dependent element-granular addressing, which Trainium2 has no fast
engine for; it is done once on the host, and the device runs the dense
pipeline.

Two exact reductions beyond the baseline:
  * Dead-unit pruning: with 80% gate sparsity, ~10.7% of hidden units
    have ALL branches masked -> their Weff row is identically zero and
    the unit contributes nothing.  Those rows/columns are dropped
    exactly (h = relu(0) = 0), shrinking both layers' weights ~19%.
  * Weights are host-packed partition-major, grouped by output-column
    chunk, so each chunk is ONE ~1-1.6MB DMA with >=4KB-per-partition
    descriptors (HBM line rate), and compute consumes chunks as they
    land instead of waiting on dozens of small semaphore-chained DMAs.

Sharding: data-parallel over batch (8 cores x 128 rows), effective
weights replicated per core, fp16 on the wire, fp32 PSUM accumulation.
No collectives.
"""

import os
import sys
import numpy as np

for _p in ("/opt/trn_rl_repo",):
    if os.path.isdir(_p) and _p not in sys.path:
        sys.path.append(_p)

from contextlib import ExitStack

from concourse import bass, mybir
import concourse.bacc as bacc
import concourse.tile as tile
from concourse.bass_utils import run_bass_kernel_spmd
from concourse.masks import make_identity

F32 = mybir.dt.float32
F16 = mybir.dt.float16

BATCH, NIN, NH, NOUT = 1024, 784, 2000, 10
NCORES = 8
BS = BATCH // NCORES            # 128 batch rows per core
P = 128
KT1 = 6                         # full 128-row L1 k-tiles (rows 0..767)
K1 = P
KTAIL = NIN - KT1 * P           # 16-row k-tail, loaded as its own tiny tile

# Exposed for the test harness: the BassKernelResults of the last run.
LAST_RESULT = None
_CACHE = {}


def _chunks(n_cols, first):
    """Output-column chunks: small first chunk (starts PE early), the
    rest 512 wide (one PSUM bank), all multiples of 128."""
    assert n_cols % 128 == 0
    out = []
    first = min(first, n_cols)
    out.append((0, first))
    off = first
    while off < n_cols:
        w = min(512, n_cols - off)
        out.append((off, w))
        off += w
    return out


def _build_weff(w, idx, mask_row, n_in):
    """Fold masks + branch sum into a dense effective weight matrix.

    Weff[o, i] = sum_{r,k} mask_row[r, o] * w[r, o, k] * [idx[r, o, k] == i]
    """
    n_br, n_out, npb = w.shape
    acc = np.zeros(n_out * n_in, np.float64)
    base = (np.arange(n_out, dtype=np.int64) * n_in)[:, None]
    for r in range(n_br):
        flat = (base + idx[r].astype(np.int64)).ravel()
        vals = (w[r].astype(np.float64) * mask_row[r].astype(np.float64)[:, None]).ravel()
        acc += np.bincount(flat, weights=vals, minlength=n_out * n_in)
    return acc.reshape(n_out, n_in).astype(np.float32)


def _pack_chunks(wt, kt, kp, chunks):
    """Pack wt (kt*kp rows, n_cols) into the on-wire layout:
    flat[p, chunk-major: (c, t, col)] = wt[t*kp+p, c0+col], so one chunk
    is per-partition contiguous (kt * w * 2 bytes)."""
    n_cols = wt.shape[1]
    total = kt * n_cols
    out = np.zeros((kp, total), np.float16)
    pos = 0
    for (c0, w) in chunks:
        blk = wt[:, c0:c0 + w].reshape(kt, kp, w)     # [t, p, col]
        out[:, pos:pos + kt * w] = blk.transpose(1, 0, 2).reshape(kp, kt * w)
        pos += kt * w
    return out


def _mlp_body(tc, n1t, n2t, xT, xTl, w1pk, w1tlk, w2pk, w3p, b3r, out):
    nc = tc.nc
    n1, n2 = n1t * P, n2t * P
    ch1 = _chunks(n1, 512)
    ch2 = _chunks(n2, 512)

    with ExitStack() as ctx:
        const = ctx.enter_context(tc.tile_pool(name="const", bufs=1))
        wp = ctx.enter_context(tc.tile_pool(name="wslab", bufs=1))
        act = ctx.enter_context(tc.tile_pool(name="act", bufs=1))
        pacc = ctx.enter_context(tc.tile_pool(name="pacc", bufs=1, space="PSUM"))
        ptr = ctx.enter_context(tc.tile_pool(name="ptr", bufs=1, space="PSUM"))

        ident = const.tile([P, P], F16, tag="ident")
        warmt = const.tile([P, P], F16, tag="warmt")
        nc.gpsimd.memset(warmt[:], 0.0)   # ready ~2.5us before the ident iota
        make_identity(nc, ident[:])

        # Full-128-partition DMAs only (partition-split halves the SDMA
        # engine set).  Items alternate between the two HWDGE rings in
        # exact consumption order, so each ring's FIFO delivers in order
        # and the rings stay byte-balanced (they round-robin at packet
        # granularity, draining at ~equal rates).
        # Single queue for the whole weight stream: one InstDMACopy
        # already fans across all 16 SDMA engines, and a single FIFO
        # delivers in exact consumption order with zero cross-queue skew.
        def ordered_dma(sbuf_ap, dram_ap):
            nc.sync.dma_start(out=sbuf_ap, in_=dram_ap)

        # x first: stationary operand of every L1 matmul.
        xbig = const.tile([K1, KT1, BS], F16, tag="xbig")
        nc.scalar.dma_start(out=xbig[:], in_=xT)
        xts = [xbig[:, t, :] for t in range(KT1)]
        xtl = const.tile([KTAIL, BS], F16, tag="xtl")
        nc.scalar.dma_start(out=xtl[:], in_=xTl)
        w1tl = const.tile([KTAIL, n1], F16, tag="w1tl")
        nc.scalar.dma_start(out=w1tl[:], in_=w1tlk)

        b3t = const.tile([NOUT, 1], F32, tag="b3")
        nc.gpsimd.dma_start(out=b3t[:], in_=b3r)
        w3t = const.tile([P, n2t, NOUT], F16, tag="w3")
        nc.gpsimd.dma_start(out=w3t[:], in_=w3p)

        # Weight chunk slabs, issued in exact consumption order.  w2
        # chunks are split into two k-halves so the PE's per-chunk wait
        # stays well under the HAM re-throttle window.
        w1s, pos = [], 0
        for i, (c0, w) in enumerate(ch1):
            slab = wp.tile([K1, KT1, w], F16, name=f"w1s{i}", tag=f"w1s{i}")
            ordered_dma(slab[:], w1pk[:, pos:pos + KT1 * w])
            w1s.append(slab)
            pos += KT1 * w
        # w2 chunks arrive as separate k-sub-TILES.  Tile tracks
        # dependencies per tile, so a consumer matmul waits for every
        # DMA that writes its tile -- sub-chunks must be distinct tiles
        # for the t-loop to chase the stream.
        w2s, pos = [], 0
        for i, (c0, w) in enumerate(ch2):
            nsplit = 2
            bounds = [round(s * n1t / nsplit) for s in range(nsplit + 1)]
            subs = []
            for s in range(nsplit):
                k0, k1 = bounds[s], bounds[s + 1]
                st = wp.tile([P, k1 - k0, w], F16,
                             name=f"w2s{i}_{s}", tag=f"w2s{i}_{s}")
                ordered_dma(st[:], w2pk[:, pos + k0 * w:pos + k1 * w])
                subs.append((st, k0, k1))
            w2s.append(subs)
            pos += n1t * w

        def w2rhs(i, t):
            for st, k0, k1 in w2s[i]:
                if k0 <= t < k1:
                    return st[:, t - k0, :]
            raise IndexError

        # PE warmup: a long uninterrupted burst of dummy matmuls so the
        # HAM clock gate sees a full busy window and unthrottles the PE
        # to 2.4 GHz before the first weight chunk lands.  The DMA
        # stream is the long pole, so this costs nothing end-to-end.
        pts = [ptr.tile([P, P], F16, name=f"pt{i}", tag=f"pt{i}")
               for i in range(2)]
        psw = pacc.tile([P, P], F32, tag="psw")
        def keepalive(n):
            for _ in range(n):
                nc.tensor.matmul(psw[:], lhsT=warmt[:], rhs=warmt[:],
                                 start=True, stop=True)
        keepalive(48)

        # Activations: one tile per chunk / per k-tile.  (Dependencies
        # are per-tile: a single h1Tb tile would make every L2 matmul
        # wait on ALL 14 transpose-copies.)
        h1c = [act.tile([P, w], F16, name=f"h1c{i}", tag=f"h1c{i}")
               for i, (_, w) in enumerate(ch1)]
        h1T = [act.tile([P, P], F16, name=f"h1T{t}", tag=f"h1T{t}")
               for t in range(n1t)]
        h2r = [act.tile([P, w], F16, name=f"h2r{i}", tag=f"h2r{i}")
               for i, (_, w) in enumerate(ch2)]
        h2T = [act.tile([P, P], F16, name=f"h2T{t}", tag=f"h2T{t}")
               for t in range(n2t)]
        ps3 = ptr.tile([NOUT, P], F32, tag="ps3")
        o = act.tile([NOUT, P], F32, tag="o")
        pti = 0

        # Deferred per-chunk epilogues.  Emitting chunk i's transposes
        # AFTER chunk i+1's matmuls keeps the PE's strict in-order queue
        # from head-of-line blocking on DVE/ACT relu+copy latency.
        def l1_epilogue(i):
            # relus live on ACT only; copies on DVE only -- otherwise a
            # DVE copy waiting on a PE transpose blocks the next relu,
            # which blocks the next transpose (mutual serialization).
            nc.scalar.activation(h1c[i][:], ps1[i][:],
                                 mybir.ActivationFunctionType.Relu)

        def l1_transposes(i):
            nonlocal pti
            c0, w = ch1[i]
            for j in range(w // P):
                jg = (c0 // P) + j
                pt = pts[pti % 2]; pti += 1
                nc.tensor.transpose(pt[:], h1c[i][:, j * P:(j + 1) * P], ident[:])
                nc.vector.tensor_copy(h1T[jg][:], pt[:])

        def l2_relu(i):
            nc.scalar.activation(h2r[i][:], ps2[i][:],
                                 mybir.ActivationFunctionType.Relu)

        def l2_tail(i):
            nonlocal pti
            c0, w = ch2[i]
            for j in range(w // P):
                jg = (c0 // P) + j
                pt = pts[pti % 2]; pti += 1
                nc.tensor.transpose(pt[:], h2r[i][:, j * P:(j + 1) * P], ident[:])
                nc.vector.tensor_copy(h2T[jg][:], pt[:])
                nc.tensor.matmul(ps3[:], lhsT=w3t[:, jg, :], rhs=h2T[jg][:],
                                 start=(jg == 0), stop=(jg == n2t - 1))

        # ---- Layer 1, software-pipelined by one chunk
        ps1 = [pacc.tile([P, w], F32, name=f"ps1_{i}", tag=f"ps{i % 2}")
               for i, (_, w) in enumerate(ch1)]
        for i in range(len(ch1)):
            c0, w = ch1[i]
            for t in range(KT1):
                nc.tensor.matmul(ps1[i][:], lhsT=xts[t], rhs=w1s[i][:, t, :],
                                 start=(t == 0), stop=False)
            nc.tensor.matmul(ps1[i][:], lhsT=xtl[:], rhs=w1tl[:, c0:c0 + w],
                             start=False, stop=True)
            l1_epilogue(i)
            if i >= 1:
                l1_transposes(i - 1)
        l1_transposes(len(ch1) - 1)

        # ---- Layer 2, software-pipelined by one chunk (h2 transposes +
        # per-tile L3 accumulation trail by one chunk)
        ps2 = [pacc.tile([P, w], F32, name=f"ps2_{i}", tag=f"ps{2 + i % 2}")
               for i, (_, w) in enumerate(ch2)]
        n2c = len(ch2)
        for i in range(n2c):
            for t in range(n1t):
                nc.tensor.matmul(ps2[i][:], lhsT=h1T[t][:], rhs=w2rhs(i, t),
                                 start=(t == 0), stop=(t == n1t - 1))
            l2_relu(i)
            if i >= 1:
                l2_tail(i - 1)
        l2_tail(n2c - 1)

        nc.vector.tensor_add(o[:], ps3[:], b3t[:].to_broadcast([NOUT, P]))
        nc.sync.dma_start(out=out, in_=o[:])


def _get_program(n1t, n2t):
    key = (n1t, n2t)
    if key in _CACHE:
        return _CACHE[key]
    nc = bacc.Bacc("TRN2", target_bir_lowering=False, debug=False,
                   enable_asserts=False, enable_partition_id=False,
                   num_devices=NCORES)
    n1, n2 = n1t * P, n2t * P
    xT = nc.dram_tensor("xT", [K1, KT1, BS], F16, kind="ExternalInput").ap()
    xTl = nc.dram_tensor("xTl", [KTAIL, BS], F16, kind="ExternalInput").ap()
    w1pk = nc.dram_tensor("w1pk", [K1, KT1 * n1], F16, kind="ExternalInput").ap()
    w1tlk = nc.dram_tensor("w1tlk", [KTAIL, n1], F16, kind="ExternalInput").ap()
    w2pk = nc.dram_tensor("w2pk", [P, n1t * n2], F16, kind="ExternalInput").ap()
    w3p = nc.dram_tensor("w3p", [P, n2t, NOUT], F16, kind="ExternalInput").ap()
    b3r = nc.dram_tensor("b3r", [NOUT, 1], F32, kind="ExternalInput").ap()
    out = nc.dram_tensor("out", [NOUT, BS], F32, kind="ExternalOutput").ap()
    with tile.TileContext(nc) as tc:
        _mlp_body(tc, n1t, n2t, xT, xTl, w1pk, w1tlk, w2pk, w3p, b3r, out)
    nc.compile()
    _CACHE[key] = nc
    return nc


def kernel(x, w1, idx1, w2, idx2, masks1, masks2, W3, b3, context):
    global LAST_RESULT
    x = np.ascontiguousarray(np.asarray(x, dtype=np.float32))
    ctxi = int(np.asarray(context))

    weff1 = _build_weff(np.asarray(w1), np.asarray(idx1),
                        np.asarray(masks1)[ctxi], NIN)
    weff2 = _build_weff(np.asarray(w2), np.asarray(idx2),
                        np.asarray(masks2)[ctxi], NH)

    # Exact dead-unit pruning: units whose Weff row is identically zero
    # output relu(0)=0 and contribute nothing downstream.
    a1 = np.flatnonzero(np.abs(weff1).sum(1))
    a2 = np.flatnonzero(np.abs(weff2).sum(1))
    n1t = max(1, -(-len(a1) // P))
    n2t = max(1, -(-len(a2) // P))
    n1, n2 = n1t * P, n2t * P

    w1p = np.zeros((n1, NIN), np.float32); w1p[:len(a1)] = weff1[a1]
    w2p = np.zeros((n2, n1), np.float32)
    w2p[:len(a2), :len(a1)] = weff2[np.ix_(a2, a1)]
    W3p = np.zeros((n2, NOUT), np.float32); W3p[:len(a2)] = np.asarray(W3)[a2]

    # w1 transposed to (n_in, n1); rows 0..767 stream chunked, the
    # 16-row k-tail ships as its own tiny early tensor (no zero pad)
    w1t = np.ascontiguousarray(w1p.T)                      # (784, n1)
    w2t = w2p.T                                            # (n1, n2)

    w1pk = _pack_chunks(w1t[:KT1 * K1].astype(np.float16), KT1, K1,
                        _chunks(n1, 512))
    w1tlk = np.ascontiguousarray(w1t[KT1 * K1:].astype(np.float16))
    w2pk = _pack_chunks(w2t.astype(np.float16), n1t, P, _chunks(n2, 512))

    w3p = np.zeros((P, n2t, NOUT), np.float16)
    w3p[:, :, :] = W3p.astype(np.float16).reshape(n2t, P, NOUT).transpose(1, 0, 2)
    b3r = np.ascontiguousarray(
        np.asarray(b3, dtype=np.float32).reshape(NOUT, 1))

    try:
        import antenv.axon_hooks  # noqa: F401
    except Exception:
        os.environ.setdefault("BASS_NEVER_TRACE", "1")

    nc = _get_program(n1t, n2t)
    in_maps = []
    for c in range(NCORES):
        xs = x[c * BS:(c + 1) * BS].T.astype(np.float16)   # (784, 128)
        xT = np.ascontiguousarray(
            xs[:KT1 * K1].reshape(KT1, K1, BS).transpose(1, 0, 2))
        xTl = np.ascontiguousarray(xs[KT1 * K1:])          # (16, 128)
        in_maps.append({"xT": xT, "xTl": xTl, "w1pk": w1pk, "w1tlk": w1tlk,
                        "w2pk": w2pk, "w3p": w3p, "b3r": b3r})

    LAST_RESULT = run_bass_kernel_spmd(nc, in_maps, list(range(NCORES)))
    return np.concatenate(
        [LAST_RESULT.results[c]["out"].T for c in range(NCORES)], axis=0)
